# revision 2
# baseline (speedup 1.0000x reference)
"""AttentionFusion kernel for 8 TRN2 NeuronCores.

Reference computation:
    expanded_video = repeat_interleave(video, 20, dim=1)        # [B, 1280, D]
    scores = expanded_video @ text.T * D**-0.5                  # [B, 1280, 256]
    attn_out = softmax(scores) @ text                           # [B, 1280, D]
    out = concat([text, video, expanded_video + attn_out], 1)   # [B, 1600, D]

Key algebraic fact: repeated (identical) query rows produce identical
attention outputs, so only the 64 unique video rows per batch need
attention; the 20x replication happens on the host during unsharding.

Sharding: B=4 batches x 2 query-halves -> 8 cores, 32 queries each.
No collectives; each core is independent.

Per-core device work (all matmuls in bf16, accumulation fp32):
  stage 1: S[32, 256]  = sum_j QT_j.T @ TT_j   (j: 80 chunks of 128 d-vals)
  softmax: W = softmax(S * SCALE, axis=k)  (max/exp+sum/recip on DVE+ACT)
  transpose: WT[256, 32] via TensorE transpose (2 x 128-blocks)
  stage 2: O.[q, d] = sum_kt WT_kt.T @ TN_kt, 4x column-tiled so the
           output lands on 128 partitions ([128, 2560] scrambled layout,
           unscrambled on host).

Host pre-transposes inputs into the layouts the TensorEngine needs
(contraction dim on partitions), so every DMA is contiguous.
"""

import sys

import numpy as np

if "/opt/trn_rl_repo" not in sys.path:
    sys.path.insert(0, "/opt/trn_rl_repo")

import ml_dtypes

REPEAT = 20
D = 10240
SCALE = D ** (-0.5)
B, TT, TV = 4, 256, 64
NCORES = 8
QPC = 32          # queries per core
DJ = 80           # number of 128-wide d chunks (stage-1 contraction tiles)
KT = 2            # number of 128-wide k tiles (stage-2 contraction tiles)
NR = 5            # stage-2 rounds; each round = 4 col-tiled groups x 512 cols
TT_CHUNK = 20     # stage-1 j's per input DMA chunk

_compiled = None


def _build():
    import concourse.mybir as mybir
    import concourse.tile as tile
    from concourse import bacc
    from concourse.masks import make_identity

    f32 = mybir.dt.float32
    bf16 = mybir.dt.bfloat16

    nc = bacc.Bacc(
        "TRN2", target_bir_lowering=False, debug=False, num_devices=NCORES
    )
    qt_h = nc.dram_tensor("qt", [128, DJ, QPC], bf16, kind="ExternalInput")
    tt_h = nc.dram_tensor("tt", [128, DJ, TT], bf16, kind="ExternalInput")
    tn_h = nc.dram_tensor("tn", [128, KT, D], bf16, kind="ExternalInput")
    out_h = nc.dram_tensor("out", [128, NR * 512], f32, kind="ExternalOutput")

    with tile.TileContext(nc) as tc:
        with (
            tc.tile_pool(name="qtp", bufs=1) as qt_pool,
            tc.tile_pool(name="ttp", bufs=4) as tt_pool,
            tc.tile_pool(name="tnp", bufs=NR) as tn_pool,
            tc.tile_pool(name="smp", bufs=1) as sm_pool,
            tc.tile_pool(name="osp", bufs=2) as os_pool,
            tc.tile_pool(name="ps_s", bufs=1, space="PSUM") as ps_s_pool,
            tc.tile_pool(name="ps_w", bufs=2, space="PSUM") as ps_w_pool,
            tc.tile_pool(name="ps_o", bufs=2, space="PSUM") as ps_o_pool,
        ):
            ident = sm_pool.tile([QPC, QPC], bf16, tag="ident")
            make_identity(nc, ident[:])

            qt_sb = qt_pool.tile([128, DJ, QPC], bf16)
            nc.sync.dma_start(qt_sb[:], qt_h[:])

            # stage 1: S = Q @ T.T, contraction over d on partitions
            ps_s = ps_s_pool.tile([QPC, TT], f32)
            for c in range(DJ // TT_CHUNK):
                tt_sb = tt_pool.tile([128, TT_CHUNK, TT], bf16)
                nc.sync.dma_start(
                    tt_sb[:], tt_h[:, c * TT_CHUNK : (c + 1) * TT_CHUNK, :]
                )
                for j in range(TT_CHUNK):
                    jj = c * TT_CHUNK + j
                    nc.tensor.matmul(
                        ps_s[:],
                        lhsT=qt_sb[:, jj, :],
                        rhs=tt_sb[:, j, :],
                        start=(jj == 0),
                        stop=(jj == DJ - 1),
                    )

            # stage-2 operand streams in while stage 1 runs
            tn_sb = []
            for r in range(NR):
                t = tn_pool.tile([128, KT, 2048], bf16)
                nc.sync.dma_start(t[:], tn_h[:, :, r * 2048 : (r + 1) * 2048])
                tn_sb.append(t)

            # softmax along k (the free dim of ps_s)
            mx = sm_pool.tile([QPC, 1], f32, tag="mx")
            nc.vector.reduce_max(mx[:], ps_s[:], axis=mybir.AxisListType.X)
            negb = sm_pool.tile([QPC, 1], f32, tag="negb")
            nc.scalar.mul(negb[:], mx[:], -SCALE)
            e = sm_pool.tile([QPC, TT], f32, tag="e")
            lsum = sm_pool.tile([QPC, 1], f32, tag="lsum")
            nc.scalar.activation(
                e[:],
                ps_s[:],
                mybir.ActivationFunctionType.Exp,
                bias=negb[:],
                scale=SCALE,
                accum_out=lsum[:],
            )
            rl = sm_pool.tile([QPC, 1], f32, tag="rl")
            nc.vector.reciprocal(rl[:], lsum[:])
            w = sm_pool.tile([QPC, TT], bf16, tag="w")
            nc.vector.tensor_scalar_mul(w[:], e[:], rl[:])

            # W[32, 256] -> WT[128, 2, 32] (k on partitions) via PE transpose
            wt_sb = sm_pool.tile([128, KT, QPC], bf16, tag="wt")
            for kt in range(KT):
                wt_ps = ps_w_pool.tile([128, QPC], bf16)
                nc.tensor.transpose(
                    wt_ps[:], w[:, kt * 128 : (kt + 1) * 128], ident[:]
                )
                nc.scalar.copy(wt_sb[:, kt, :], wt_ps[:])

            # stage 2: O = W @ T, 4x column-tiled; group g writes psum
            # partitions [32g, 32g+32) so output uses all 128 partitions
            for r in range(NR):
                ps_o = ps_o_pool.tile([128, 512], f32)
                for g in range(4):
                    for kt in range(KT):
                        nc.tensor.matmul(
                            ps_o[g * QPC : (g + 1) * QPC, :],
                            lhsT=wt_sb[:, kt, :],
                            rhs=tn_sb[r][:, kt, g * 512 : (g + 1) * 512],
                            start=(kt == 0),
                            stop=(kt == KT - 1),
                            tile_position=(0, g * QPC),
                        )
                osb = os_pool.tile([128, 512], f32)
                nc.scalar.copy(osb[:], ps_o[:])
                nc.sync.dma_start(out_h[:, r * 512 : (r + 1) * 512], osb[:])

    nc.compile()
    return nc


def _prepare_in_maps(text, video):
    tb = np.asarray(text, dtype=np.float32).astype(ml_dtypes.bfloat16)
    vb = np.asarray(video, dtype=np.float32).astype(ml_dtypes.bfloat16)
    in_maps = []
    for c in range(NCORES):
        b, h = divmod(c, 2)
        # tt[p, j, k] = text[b, k, j*128+p]
        tt = np.ascontiguousarray(tb[b].reshape(TT, DJ, 128).transpose(2, 1, 0))
        # tn[p, kt, d] = text[b, kt*128+p, d]
        tn = np.ascontiguousarray(tb[b].reshape(KT, 128, D).transpose(1, 0, 2))
        # qt[p, j, q] = video[b, h*32+q, j*128+p]
        qt = np.ascontiguousarray(
            vb[b, h * QPC : (h + 1) * QPC].reshape(QPC, DJ, 128).transpose(2, 1, 0)
        )
        in_maps.append({"qt": qt, "tt": tt, "tn": tn})
    return in_maps


def _assemble(results, text, video):
    tf = np.asarray(text, dtype=np.float32)
    vf = np.asarray(video, dtype=np.float32)
    attn = np.empty((B, TV, D), np.float32)
    for c in range(NCORES):
        b, h = divmod(c, 2)
        o128 = np.asarray(results[c]["out"], dtype=np.float32)
        # out128[32g+q, r*512+x] = O[q, (r*4+g)*512+x]
        o = o128.reshape(4, QPC, NR, 512).transpose(1, 2, 0, 3).reshape(QPC, D)
        attn[b, h * QPC : (h + 1) * QPC] = o
    fused = vf + attn
    return np.concatenate([tf, vf, np.repeat(fused, REPEAT, axis=1)], axis=1)


def _ensure_ntff_hook():
    """Register the axon NTFF profiling hook if the image lacks
    antenv.axon_hooks (trace=True degrades to no-op otherwise)."""
    import types

    try:
        from antenv import axon_hooks  # noqa: F401

        return
    except ImportError:
        pass
    mod = types.ModuleType("antenv.axon_hooks")
    _hook = [None]
    mod.set_axon_ntff_profile_hook = lambda h: _hook.__setitem__(0, h)
    mod.get_axon_ntff_profile_hook = lambda: _hook[0]
    sys.modules["antenv.axon_hooks"] = mod
    import antenv

    antenv.axon_hooks = mod
    try:
        from trn_agent_boot.trn_boot import _ntff_profile_via_ctypes

        mod.set_axon_ntff_profile_hook(
            _ntff_profile_via_ctypes("/opt/axon/libaxon_pjrt.so")
        )
    except Exception:
        pass


def _run(text_features, video_features, trace=False, **spmd_kwargs):
    global _compiled
    if _compiled is None:
        _compiled = _build()
    if trace:
        _ensure_ntff_hook()
    from concourse.bass_utils import run_bass_kernel_spmd

    in_maps = _prepare_in_maps(text_features, video_features)
    res = run_bass_kernel_spmd(
        _compiled,
        in_maps,
        core_ids=list(range(NCORES)),
        trace=trace,
        **spmd_kwargs,
    )
    out = _assemble(res.results, text_features, video_features)
    return out, res


def kernel(text_features, video_features):
    out, _ = _run(text_features, video_features)
    return out


# revision 3
# speedup vs baseline: 1.1325x; 1.1325x over previous
"""AttentionFusion kernel for 8 TRN2 NeuronCores.

Reference computation:
    expanded_video = repeat_interleave(video, 20, dim=1)        # [B, 1280, D]
    scores = expanded_video @ text.T * D**-0.5                  # [B, 1280, 256]
    attn_out = softmax(scores) @ text                           # [B, 1280, D]
    out = concat([text, video, expanded_video + attn_out], 1)   # [B, 1600, D]

Key algebraic fact: repeated (identical) query rows produce identical
attention outputs, so only the 64 unique video rows per batch need
attention; the 20x replication happens on the host during unsharding.

Sharding: B=4 batches x 2 query-halves -> 8 cores, 32 queries each.
No collectives; each core is independent.

Per-core device work (all matmuls in bf16, accumulation fp32):
  stage 1: S[32, 256]  = sum_j QT_j.T @ TT_j   (j: 80 chunks of 128 d-vals)
  softmax: W = softmax(S * SCALE, axis=k)  (max/exp+sum/recip on DVE+ACT)
  transpose: WT[256, 32] via TensorE transpose (2 x 128-blocks)
  stage 2: O.[q, d] = sum_kt WT_kt.T @ TN_kt, 4x column-tiled so the
           output lands on 128 partitions ([128, 2560] scrambled layout,
           unscrambled on host).

Host pre-transposes inputs into the layouts the TensorEngine needs
(contraction dim on partitions), so every DMA is contiguous.
"""

import sys

import numpy as np

if "/opt/trn_rl_repo" not in sys.path:
    sys.path.insert(0, "/opt/trn_rl_repo")

import ml_dtypes

REPEAT = 20
D = 10240
SCALE = D ** (-0.5)
B, TT, TV = 4, 256, 64
NCORES = 8
QPC = 32          # queries per core
DJ = 80           # number of 128-wide d chunks (stage-1 contraction tiles)
KT = 2            # number of 128-wide k tiles (stage-2 contraction tiles)
NR = 5            # stage-2 rounds; each round = 4 col-tiled groups x 512 cols
TT_CHUNK = 20     # stage-1 j's per input DMA chunk

_compiled = None


def _build():
    import concourse.mybir as mybir
    import concourse.tile as tile
    from concourse import bacc
    from concourse.masks import make_identity

    f32 = mybir.dt.float32
    bf16 = mybir.dt.bfloat16
    fp8 = mybir.dt.float8e3

    nc = bacc.Bacc(
        "TRN2", target_bir_lowering=False, debug=False, num_devices=NCORES
    )
    qt_h = nc.dram_tensor("qt", [128, DJ, QPC], fp8, kind="ExternalInput")
    tt_h = nc.dram_tensor("tt", [128, DJ, TT], fp8, kind="ExternalInput")
    tn_h = nc.dram_tensor("tn", [128, KT, D], bf16, kind="ExternalInput")
    out_h = nc.dram_tensor("out", [128, NR * 512], bf16, kind="ExternalOutput")

    with tile.TileContext(nc) as tc:
        with (
            tc.tile_pool(name="qtp", bufs=1) as qt_pool,
            tc.tile_pool(name="ttp", bufs=4) as tt_pool,
            tc.tile_pool(name="tnp", bufs=NR) as tn_pool,
            tc.tile_pool(name="smp", bufs=1) as sm_pool,
            tc.tile_pool(name="osp", bufs=2) as os_pool,
            tc.tile_pool(name="ps_s", bufs=1, space="PSUM") as ps_s_pool,
            tc.tile_pool(name="ps_w", bufs=2, space="PSUM") as ps_w_pool,
            tc.tile_pool(name="ps_o", bufs=2, space="PSUM") as ps_o_pool,
        ):
            ident = sm_pool.tile([QPC, QPC], bf16, tag="ident")
            make_identity(nc, ident[:])

            qt_sb = qt_pool.tile([128, DJ, QPC], fp8)
            nc.sync.dma_start(qt_sb[:], qt_h[:])

            # stage 1: S = Q @ T.T, contraction over d on partitions
            ps_s = ps_s_pool.tile([QPC, TT], f32)
            for c in range(DJ // TT_CHUNK):
                tt_sb = tt_pool.tile([128, TT_CHUNK, TT], fp8)
                nc.sync.dma_start(
                    tt_sb[:], tt_h[:, c * TT_CHUNK : (c + 1) * TT_CHUNK, :]
                )
                for j in range(TT_CHUNK):
                    jj = c * TT_CHUNK + j
                    nc.tensor.matmul(
                        ps_s[:],
                        lhsT=qt_sb[:, jj, :],
                        rhs=tt_sb[:, j, :],
                        start=(jj == 0),
                        stop=(jj == DJ - 1),
                    )

            # stage-2 operand streams in while stage 1 runs
            tn_sb = []
            for r in range(NR):
                t = tn_pool.tile([128, KT, 2048], bf16)
                nc.sync.dma_start(t[:], tn_h[:, :, r * 2048 : (r + 1) * 2048])
                tn_sb.append(t)

            # softmax along k (the free dim of ps_s)
            mx = sm_pool.tile([QPC, 1], f32, tag="mx")
            nc.vector.reduce_max(mx[:], ps_s[:], axis=mybir.AxisListType.X)
            negb = sm_pool.tile([QPC, 1], f32, tag="negb")
            nc.scalar.mul(negb[:], mx[:], -SCALE)
            e = sm_pool.tile([QPC, TT], f32, tag="e")
            lsum = sm_pool.tile([QPC, 1], f32, tag="lsum")
            nc.scalar.activation(
                e[:],
                ps_s[:],
                mybir.ActivationFunctionType.Exp,
                bias=negb[:],
                scale=SCALE,
                accum_out=lsum[:],
            )
            rl = sm_pool.tile([QPC, 1], f32, tag="rl")
            nc.vector.reciprocal(rl[:], lsum[:])
            w = sm_pool.tile([QPC, TT], bf16, tag="w")
            nc.vector.tensor_scalar_mul(w[:], e[:], rl[:])

            # W[32, 256] -> WT[128, 2, 32] (k on partitions) via PE transpose
            wt_sb = sm_pool.tile([128, KT, QPC], bf16, tag="wt")
            for kt in range(KT):
                wt_ps = ps_w_pool.tile([128, QPC], bf16)
                nc.tensor.transpose(
                    wt_ps[:], w[:, kt * 128 : (kt + 1) * 128], ident[:]
                )
                nc.scalar.copy(wt_sb[:, kt, :], wt_ps[:])

            # stage 2: O = W @ T, 4x column-tiled; group g writes psum
            # partitions [32g, 32g+32) so output uses all 128 partitions
            for r in range(NR):
                ps_o = ps_o_pool.tile([128, 512], f32)
                for g in range(4):
                    for kt in range(KT):
                        nc.tensor.matmul(
                            ps_o[g * QPC : (g + 1) * QPC, :],
                            lhsT=wt_sb[:, kt, :],
                            rhs=tn_sb[r][:, kt, g * 512 : (g + 1) * 512],
                            start=(kt == 0),
                            stop=(kt == KT - 1),
                            tile_position=(0, g * QPC),
                        )
                osb = os_pool.tile([128, 512], bf16)
                nc.scalar.copy(osb[:], ps_o[:])
                nc.sync.dma_start(out_h[:, r * 512 : (r + 1) * 512], osb[:])

    nc.compile()
    return nc


def _prepare_in_maps(text, video):
    tb = np.asarray(text, dtype=np.float32).astype(ml_dtypes.bfloat16)
    t8 = np.asarray(text, dtype=np.float32).astype(ml_dtypes.float8_e3m4)
    v8 = np.asarray(video, dtype=np.float32).astype(ml_dtypes.float8_e3m4)
    in_maps = []
    for c in range(NCORES):
        b, h = divmod(c, 2)
        # tt[p, j, k] = text[b, k, j*128+p]
        tt = np.ascontiguousarray(t8[b].reshape(TT, DJ, 128).transpose(2, 1, 0))
        # tn[p, kt, d] = text[b, kt*128+p, d]
        tn = np.ascontiguousarray(tb[b].reshape(KT, 128, D).transpose(1, 0, 2))
        # qt[p, j, q] = video[b, h*32+q, j*128+p]
        qt = np.ascontiguousarray(
            v8[b, h * QPC : (h + 1) * QPC].reshape(QPC, DJ, 128).transpose(2, 1, 0)
        )
        in_maps.append({"qt": qt, "tt": tt, "tn": tn})
    return in_maps


def _assemble(results, text, video):
    tf = np.asarray(text, dtype=np.float32)
    vf = np.asarray(video, dtype=np.float32)
    attn = np.empty((B, TV, D), np.float32)
    for c in range(NCORES):
        b, h = divmod(c, 2)
        o128 = np.asarray(results[c]["out"], dtype=np.float32)
        # out128[32g+q, r*512+x] = O[q, (r*4+g)*512+x]
        o = o128.reshape(4, QPC, NR, 512).transpose(1, 2, 0, 3).reshape(QPC, D)
        attn[b, h * QPC : (h + 1) * QPC] = o
    fused = vf + attn
    return np.concatenate([tf, vf, np.repeat(fused, REPEAT, axis=1)], axis=1)


def _ensure_ntff_hook():
    """Register the axon NTFF profiling hook if the image lacks
    antenv.axon_hooks (trace=True degrades to no-op otherwise)."""
    import types

    try:
        from antenv import axon_hooks  # noqa: F401

        return
    except ImportError:
        pass
    mod = types.ModuleType("antenv.axon_hooks")
    _hook = [None]
    mod.set_axon_ntff_profile_hook = lambda h: _hook.__setitem__(0, h)
    mod.get_axon_ntff_profile_hook = lambda: _hook[0]
    sys.modules["antenv.axon_hooks"] = mod
    import antenv

    antenv.axon_hooks = mod
    try:
        from trn_agent_boot.trn_boot import _ntff_profile_via_ctypes

        mod.set_axon_ntff_profile_hook(
            _ntff_profile_via_ctypes("/opt/axon/libaxon_pjrt.so")
        )
    except Exception:
        pass


def _run(text_features, video_features, trace=False, **spmd_kwargs):
    global _compiled
    if _compiled is None:
        _compiled = _build()
    if trace:
        _ensure_ntff_hook()
    from concourse.bass_utils import run_bass_kernel_spmd

    in_maps = _prepare_in_maps(text_features, video_features)
    res = run_bass_kernel_spmd(
        _compiled,
        in_maps,
        core_ids=list(range(NCORES)),
        trace=trace,
        **spmd_kwargs,
    )
    out = _assemble(res.results, text_features, video_features)
    return out, res


def kernel(text_features, video_features):
    out, _ = _run(text_features, video_features)
    return out
